# revision 24
# baseline (speedup 1.0000x reference)
"""Trainium2 Bass kernel for nn_Attention_38448547234571.

Math notes (exact algebraic reductions of the reference):
- reference rows idx in [0,128): batch = idx%16, head = idx%8 = (idx%16)%8,
  so there are only 16 distinct (batch, head) pairs, replicated 8x. We
  compute the 16 distinct slices and replicate on the host.
- out = out_pre @ proj_w.T where out_pre's feature dim is 8 copies of the
  same 128-dim block, so proj_w folds to projfold[e,h] = sum_j proj_w[e, j*128+h].
- On device we compute scoreT[kk,q] = kx @ qx.T (scaled via w_qx/sqrt(H)),
  E_T = exp(scoreT + maskbias[kk]) (mask folded into the per-partition ACT
  bias; masked rows underflow to exactly 0 like the reference), and
  out_unnorm[q,e] = (kx.T-stationary AV chain) @ projfoldT. The softmax
  denominator S[q] = sum_kk E_T and the 1/S + proj_b epilogue are applied on
  the host during unsharding (both are exact rescalings).

Device layout per core (2 batches per core, 8 cores):
  inputs : kT/qT [2,1024,1024] (e-major), wk/wq [2,1024,128], pfT [128,1024],
           mb [2,128,8] (exp bias per kk-tile)
  outputs: eo [2,1024,1024] = E_T (unnormalized exp scores, [kk,q]),
           oo [2,1024,1024] = out_unnorm ([q,e])
"""
import math
import numpy as np

B, SEQ, EMB = 16, 1024, 1024
NH, H = 8, 128
NCORES = 8
BPC = B // NCORES  # batches per core
P = 128
NT = SEQ // P  # 8 tiles per 1024 dim

_compiled = {}


def _build_kernel(dt_mm_name: str):
    """Build the Bacc module. dt_mm_name: 'float32' or 'float32r' for matmuls."""
    from concourse import bacc
    import concourse.mybir as mybir
    from concourse.tile import TileContext

    F32 = mybir.dt.float32
    if dt_mm_name == "bf16":
        DTM = mybir.dt.float32r
        DTI = mybir.dt.bfloat16
    else:
        DTM = getattr(mybir.dt, dt_mm_name)
        DTI = DTM

    nc = bacc.Bacc("TRN2", target_bir_lowering=False, num_devices=NCORES)

    kT_d = nc.dram_tensor("kT", [BPC, SEQ, SEQ], DTI, kind="ExternalInput")
    qT_d = nc.dram_tensor("qT", [BPC, SEQ, SEQ], DTI, kind="ExternalInput")
    wk_d = nc.dram_tensor("wk", [BPC, P, NT * H], DTI, kind="ExternalInput")
    wq_d = nc.dram_tensor("wq", [BPC, P, NT * H], DTI, kind="ExternalInput")
    pfT_d = nc.dram_tensor("pfT", [H, EMB], F32, kind="ExternalInput")
    mb_d = nc.dram_tensor("mb", [BPC, P, NT], F32, kind="ExternalInput")
    id_d = nc.dram_tensor("ident", [P, P], F32, kind="ExternalInput")
    eo_d = nc.dram_tensor("eo", [BPC, SEQ, SEQ], F32, kind="ExternalOutput")
    oo_d = nc.dram_tensor("oo", [BPC, SEQ, SEQ], F32, kind="ExternalOutput")

    with TileContext(nc) as tc:
        with tc.tile_pool(name="sb1", bufs=1) as sb1, \
             tc.tile_pool(name="sb2", bufs=2) as sb2, \
             tc.tile_pool(name="sb4", bufs=4) as sb4, \
             tc.tile_pool(name="ps_big", bufs=2, space="PSUM") as ps_big, \
             tc.tile_pool(name="ps_av", bufs=1, space="PSUM") as ps_avp, \
             tc.tile_pool(name="ps_acc", bufs=2, space="PSUM") as ps_acc:

            ident = sb1.tile([P, P], DTM, tag="ident")
            pfT = sb1.tile([P, EMB], DTM, tag="pfT")

            for b in range(BPC):
                # ---- load inputs (small weights first, then streamed k/q) ----
                wk = sb2.tile([P, NT * H], DTI, tag="wk")
                wq = sb2.tile([P, NT * H], DTI, tag="wq")
                weng = nc.sync if b == 0 else nc.gpsimd
                weng.dma_start(wk[:], wk_d[b])
                weng.dma_start(wq[:], wq_d[b])
                mb = sb2.tile([P, NT], F32, tag="mb")
                weng.dma_start(mb[:], mb_d[b])
                kt = sb1.tile([P, NT * SEQ], DTI, tag="kt")
                qt = sb1.tile([P, NT * SEQ], DTI, tag="qt")
                for t in range(NT):
                    weng.dma_start(
                        kt[:, t * SEQ:(t + 1) * SEQ], kT_d[b, t * P:(t + 1) * P, :])
                for t in range(NT):
                    weng.dma_start(
                        qt[:, t * SEQ:(t + 1) * SEQ], qT_d[b, t * P:(t + 1) * P, :])
                if b == 0:
                    nc.gpsimd.dma_start(ident[:], id_d[:])
                    nc.gpsimd.dma_start(pfT[:], pfT_d[:])

                # ---- kxT [h, s], qxT [h, s]: accumulate over 8 e-tiles ----
                kxT = sb2.tile([P, SEQ], DTM, tag="kxT")
                qxT = sb2.tile([P, SEQ], DTM, tag="qxT")
                for dst, w, src in ((kxT, wk, kt), (qxT, wq, qt)):
                    for c in range(2):
                        ps = ps_acc.tile([P, 512], F32, tag="acc")
                        for e in range(NT):
                            nc.tensor.matmul(
                                ps[:],
                                w[:, e * H:(e + 1) * H],
                                src[:, e * SEQ + c * 512: e * SEQ + (c + 1) * 512],
                                start=(e == 0), stop=(e == NT - 1),
                            )
                        nc.vector.tensor_copy(dst[:, c * 512:(c + 1) * 512], ps[:])

                # ---- kx blocks [kk, h] via PE transpose of kxT tiles ----
                kxall = sb2.tile([P, NT * H], DTM, tag="kxall")
                for t in range(NT):
                    pst = ps_acc.tile([P, P], DTM, tag="acc")
                    nc.tensor.transpose(pst[:, :P], kxT[:, t * P:(t + 1) * P], ident[:])
                    nc.vector.tensor_copy(kxall[:, t * H:(t + 1) * H], pst[:, :P])

                # ---- score t -> exp t -> eo store + running AV accumulation ----
                ps_av = ps_avp.tile([P, SEQ], F32, tag="av")
                for t in range(NT):
                    ps = ps_big.tile([P, SEQ], F32, tag="big")
                    for c in range(2):
                        nc.tensor.matmul(
                            ps[:, c * 512:(c + 1) * 512],
                            kxT[:, t * P:(t + 1) * P],
                            qxT[:, c * 512:(c + 1) * 512],
                            start=True, stop=True,
                        )
                    et = sb1.tile([P, SEQ], DTM, tag=f"et{t}")
                    nc.scalar.activation(
                        et[:], ps[:],
                        mybir.ActivationFunctionType.Exp,
                        bias=mb[:, t:t + 1], scale=1.0,
                    )
                    nc.sync.dma_start(
                        eo_d[b, t * P:(t + 1) * P, :], et[:].bitcast(F32))
                    for c in range(2):
                        nc.tensor.matmul(
                            ps_av[:, c * 512:(c + 1) * 512],
                            kxall[:, t * H:(t + 1) * H],
                            et[:, c * 512:(c + 1) * 512],
                            start=(t == 0), stop=(t == NT - 1),
                            skip_group_check=True,
                        )

                obT = sb2.tile([P, SEQ], DTM, tag="obT")
                nc.scalar.copy(obT[:, 0:512], ps_av[:, 0:512])
                nc.vector.tensor_copy(obT[:, 512:1024], ps_av[:, 512:1024])

                # ---- proj: out_unnorm[q, e] = obT.T @ pfT ----
                for t in range(NT):
                    ps = ps_big.tile([P, SEQ], F32, tag="big")
                    for c in range(2):
                        nc.tensor.matmul(
                            ps[:, c * 512:(c + 1) * 512],
                            obT[:, t * P:(t + 1) * P],
                            pfT[:, c * 512:(c + 1) * 512],
                            start=True, stop=True,
                        )
                    osb = sb4.tile([P, SEQ], F32, tag="osb")
                    if t % 2 == 0:
                        nc.vector.tensor_copy(osb[:], ps[:])
                    else:
                        nc.scalar.copy(osb[:], ps[:])
                    nc.sync.dma_start(oo_d[b, t * P:(t + 1) * P, :], osb[:])

    nc.compile()
    return nc


def kernel(k, q, w_kx, w_qx, proj_w, proj_b, memory_len, _dt_mm="float32r", _trace=False):
    from concourse.bass_utils import run_bass_kernel_spmd

    k = np.asarray(k, dtype=np.float32)
    q = np.asarray(q, dtype=np.float32)
    w_kx = np.asarray(w_kx, dtype=np.float32)
    w_qx = np.asarray(w_qx, dtype=np.float32)
    proj_w = np.asarray(proj_w, dtype=np.float32)
    proj_b = np.asarray(proj_b, dtype=np.float32)
    memory_len = np.asarray(memory_len)

    try:
        if _dt_mm not in _compiled:
            _compiled[_dt_mm] = _build_kernel(_dt_mm)
        nc = _compiled[_dt_mm]
    except Exception:
        if _dt_mm == "float32":
            raise
        # fall back to plain fp32 matmuls if this environment rejects f32r
        _dt_mm = "float32"
        if _dt_mm not in _compiled:
            _compiled[_dt_mm] = _build_kernel(_dt_mm)
        nc = _compiled[_dt_mm]
    if _dt_mm == "bf16":
        import ml_dtypes
        cast = lambda a: a.astype(ml_dtypes.bfloat16)
    else:
        cast = lambda a: a

    # ---- host-side sharding / preprocessing ----
    scale = np.float32(1.0 / math.sqrt(H))
    kT = cast(np.ascontiguousarray(k.transpose(0, 2, 1)))    # [B, e, s]
    qT = cast(np.ascontiguousarray(q.transpose(0, 2, 1)))
    head = np.arange(B) % NH
    def sbuf_layout(w):  # [B, e, h] -> [B, p, t*h] (partition-major e-tiles)
        return np.ascontiguousarray(
            w.reshape(B, NT, P, H).transpose(0, 2, 1, 3).reshape(B, P, NT * H))
    wk = cast(sbuf_layout(w_kx[head]))
    wq = cast(sbuf_layout(w_qx[head] * scale))
    pfT = np.ascontiguousarray(proj_w.reshape(EMB, NH, H).sum(axis=1).T)  # [h, e]
    kk = np.arange(SEQ)
    bias = np.where(kk[None, :] < memory_len[:, None], 0.0, -10000.0).astype(np.float32)
    mb = np.ascontiguousarray(bias.reshape(B, NT, P).transpose(0, 2, 1))  # [B, p, t]

    in_maps = []
    for c in range(NCORES):
        s = slice(c * BPC, (c + 1) * BPC)
        in_maps.append({
            "kT": kT[s], "qT": qT[s], "wk": wk[s], "wq": wq[s],
            "pfT": pfT, "mb": mb[s], "ident": np.eye(P, dtype=np.float32),
        })

    try:
        res = run_bass_kernel_spmd(nc, in_maps, list(range(NCORES)), trace=_trace)
    except Exception:
        if _dt_mm == "float32":
            raise
        if "float32" not in _compiled:
            _compiled["float32"] = _build_kernel("float32")
        res = run_bass_kernel_spmd(_compiled["float32"], in_maps,
                                   list(range(NCORES)), trace=_trace)
    kernel.last_results = res

    # ---- host-side unshard / epilogue ----
    out = np.empty((B, SEQ, EMB), np.float32)
    attn_base = np.empty((B, SEQ, SEQ), np.float32)
    projfold = pfT.T  # [e, h]
    for c in range(NCORES):
        eo = res.results[c]["eo"]   # [BPC, kk, q]
        oo = res.results[c]["oo"]   # [BPC, q, e]
        for j in range(BPC):
            b = c * BPC + j
            if memory_len[b] == 0:
                # fully masked: reference softmax is uniform 1/SEQ
                kx = k[b] @ w_kx[b % NH]
                attn_base[b] = np.float32(1.0 / SEQ)
                ob = np.broadcast_to(kx.mean(axis=0, dtype=np.float64).astype(np.float32), (SEQ, H))
                out[b] = ob @ projfold.T + proj_b[None, :]
                continue
            E_T = eo[j]
            S = E_T.sum(axis=0)                         # [q]
            attn_base[b] = (E_T / S[None, :]).T
            out[b] = oo[j] * (np.float32(1.0) / S)[:, None] + proj_b[None, :]
    attn = np.tile(attn_base, (NH, 1, 1))
    return out, attn


# revision 27
# speedup vs baseline: 1.0942x; 1.0942x over previous
"""Trainium2 Bass kernel for nn_Attention_38448547234571.

Math notes (exact algebraic reductions of the reference):
- reference rows idx in [0,128): batch = idx%16, head = idx%8 = (idx%16)%8,
  so there are only 16 distinct (batch, head) pairs, replicated 8x. We
  compute the 16 distinct slices and replicate on the host.
- out = out_pre @ proj_w.T where out_pre's feature dim is 8 copies of the
  same 128-dim block, so proj_w folds to projfold[e,h] = sum_j proj_w[e, j*128+h].
- On device we compute scoreT[kk,q] = kx @ qx.T (scaled via w_qx/sqrt(H)),
  E_T = exp(scoreT + maskbias[kk]) (mask folded into the per-partition ACT
  bias; masked rows underflow to exactly 0 like the reference), and
  out_unnorm[q,e] = (kx.T-stationary AV chain) @ projfoldT. The softmax
  denominator S[q] = sum_kk E_T and the 1/S + proj_b epilogue are applied on
  the host during unsharding (both are exact rescalings).

Device layout per core (2 batches per core, 8 cores):
  inputs : kT/qT [2,1024,1024] (e-major), wk/wq [2,1024,128], pfT [128,1024],
           mb [2,128,8] (exp bias per kk-tile)
  outputs: eo [2,1024,1024] = E_T (unnormalized exp scores, [kk,q]),
           oo [2,1024,1024] = out_unnorm ([q,e])
"""
import math
import numpy as np

B, SEQ, EMB = 16, 1024, 1024
NH, H = 8, 128
NCORES = 8
BPC = B // NCORES  # batches per core
P = 128
NT = SEQ // P  # 8 tiles per 1024 dim

_compiled = {}


def _build_kernel(dt_mm_name: str, tiles=(NT, NT)):
    """Build the Bacc module, specialized to per-slot live kk-tile counts.

    tiles[b] = number of leading 128-row kk-tiles that can be nonzero for
    batch slot b (rows >= memory_len are exactly 0 after the exp bias, so
    trailing fully-masked tiles need neither kT columns nor score/exp/AV/eo
    work; the host fills those attn rows with exact zeros)."""
    from concourse import bacc
    import concourse.mybir as mybir
    from concourse.tile import TileContext

    F32 = mybir.dt.float32
    if dt_mm_name == "bf16":
        DTM = mybir.dt.float32r
        DTI = mybir.dt.bfloat16
    else:
        DTM = getattr(mybir.dt, dt_mm_name)
        DTI = DTM

    nc = bacc.Bacc("TRN2", target_bir_lowering=False, num_devices=NCORES)

    kT_ds = [nc.dram_tensor(f"kT{b}", [SEQ, tiles[b] * P], DTI, kind="ExternalInput")
             for b in range(BPC)]
    qT_d = nc.dram_tensor("qT", [BPC, SEQ, SEQ], DTI, kind="ExternalInput")
    wk_d = nc.dram_tensor("wk", [BPC, P, NT * H], DTI, kind="ExternalInput")
    wq_d = nc.dram_tensor("wq", [BPC, P, NT * H], DTI, kind="ExternalInput")
    pfT_d = nc.dram_tensor("pfT", [H, EMB], F32, kind="ExternalInput")
    mb_d = nc.dram_tensor("mb", [BPC, P, NT], F32, kind="ExternalInput")
    id_d = nc.dram_tensor("ident", [P, P], F32, kind="ExternalInput")
    eo_ds = [nc.dram_tensor(f"eo{b}", [tiles[b] * P, SEQ], F32, kind="ExternalOutput")
             for b in range(BPC)]
    oo_d = nc.dram_tensor("oo", [BPC, SEQ, SEQ], F32, kind="ExternalOutput")

    with TileContext(nc) as tc:
        with tc.tile_pool(name="sb1", bufs=1) as sb1, \
             tc.tile_pool(name="sb2", bufs=2) as sb2, \
             tc.tile_pool(name="sb4", bufs=4) as sb4, \
             tc.tile_pool(name="ps_big", bufs=2, space="PSUM") as ps_big, \
             tc.tile_pool(name="ps_av", bufs=1, space="PSUM") as ps_avp, \
             tc.tile_pool(name="ps_acc", bufs=2, space="PSUM") as ps_acc:

            ident = sb1.tile([P, P], DTM, tag="ident")
            pfT = sb1.tile([P, EMB], DTM, tag="pfT")

            for b in range(BPC):
                T = tiles[b]
                W = T * P          # live kk width for this slot
                kT_d = kT_ds[b]
                eo_d_b = eo_ds[b]
                # ---- load inputs (small weights first, then streamed k/q) ----
                wk = sb2.tile([P, NT * H], DTI, tag="wk")
                wq = sb2.tile([P, NT * H], DTI, tag="wq")
                weng = nc.sync if b == 0 else nc.gpsimd
                weng.dma_start(wk[:], wk_d[b])
                weng.dma_start(wq[:], wq_d[b])
                mb = sb2.tile([P, NT], F32, tag="mb")
                weng.dma_start(mb[:], mb_d[b])
                kt = sb1.tile([P, NT * W], DTI, tag="kt")
                qt = sb1.tile([P, NT * SEQ], DTI, tag="qt")
                for t in range(NT):
                    weng.dma_start(
                        kt[:, t * W:(t + 1) * W], kT_d[t * P:(t + 1) * P, :])
                for t in range(NT):
                    weng.dma_start(
                        qt[:, t * SEQ:(t + 1) * SEQ], qT_d[b, t * P:(t + 1) * P, :])
                if b == 0:
                    nc.gpsimd.dma_start(ident[:], id_d[:])
                    nc.gpsimd.dma_start(pfT[:], pfT_d[:])

                # ---- kxT [h, s<W], qxT [h, s]: accumulate over 8 e-tiles ----
                kxT = sb2.tile([P, NT * P], DTM, tag="kxT")
                qxT = sb2.tile([P, SEQ], DTM, tag="qxT")
                for dst, w, src, width in ((kxT, wk, kt, W), (qxT, wq, qt, SEQ)):
                    off = 0
                    while off < width:
                        cw = min(512, width - off)
                        ps = ps_acc.tile([P, 512], F32, tag="acc")
                        for e in range(NT):
                            nc.tensor.matmul(
                                ps[:, :cw],
                                w[:, e * H:(e + 1) * H],
                                src[:, e * width + off: e * width + off + cw],
                                start=(e == 0), stop=(e == NT - 1),
                            )
                        nc.vector.tensor_copy(dst[:, off:off + cw], ps[:, :cw])
                        off += cw

                # ---- kx blocks [kk, h] via PE transpose of kxT tiles ----
                kxall = sb2.tile([P, NT * H], DTM, tag="kxall")
                for t in range(T):
                    pst = ps_acc.tile([P, P], DTM, tag="acc")
                    nc.tensor.transpose(pst[:, :P], kxT[:, t * P:(t + 1) * P], ident[:])
                    nc.vector.tensor_copy(kxall[:, t * H:(t + 1) * H], pst[:, :P])

                # ---- score t -> exp t -> eo store + running AV accumulation ----
                ps_av = ps_avp.tile([P, SEQ], F32, tag="av")
                for t in range(T):
                    ps = ps_big.tile([P, SEQ], F32, tag="big")
                    for c in range(2):
                        nc.tensor.matmul(
                            ps[:, c * 512:(c + 1) * 512],
                            kxT[:, t * P:(t + 1) * P],
                            qxT[:, c * 512:(c + 1) * 512],
                            start=True, stop=True,
                        )
                    et = sb1.tile([P, SEQ], DTM, tag=f"et{t}")
                    nc.scalar.activation(
                        et[:], ps[:],
                        mybir.ActivationFunctionType.Exp,
                        bias=mb[:, t:t + 1], scale=1.0,
                    )
                    nc.sync.dma_start(
                        eo_d_b[t * P:(t + 1) * P, :], et[:].bitcast(F32))
                    for c in range(2):
                        nc.tensor.matmul(
                            ps_av[:, c * 512:(c + 1) * 512],
                            kxall[:, t * H:(t + 1) * H],
                            et[:, c * 512:(c + 1) * 512],
                            start=(t == 0), stop=(t == T - 1),
                            skip_group_check=True,
                        )

                obT = sb2.tile([P, SEQ], DTM, tag="obT")
                nc.scalar.copy(obT[:, 0:512], ps_av[:, 0:512])
                nc.vector.tensor_copy(obT[:, 512:1024], ps_av[:, 512:1024])

                # ---- proj: out_unnorm[q, e] = obT.T @ pfT ----
                for t in range(NT):
                    ps = ps_big.tile([P, SEQ], F32, tag="big")
                    for c in range(2):
                        nc.tensor.matmul(
                            ps[:, c * 512:(c + 1) * 512],
                            obT[:, t * P:(t + 1) * P],
                            pfT[:, c * 512:(c + 1) * 512],
                            start=True, stop=True,
                        )
                    osb = sb4.tile([P, SEQ], F32, tag="osb")
                    if t % 2 == 0:
                        nc.vector.tensor_copy(osb[:], ps[:])
                    else:
                        nc.scalar.copy(osb[:], ps[:])
                    nc.sync.dma_start(oo_d[b, t * P:(t + 1) * P, :], osb[:])

    nc.compile()
    return nc


def kernel(k, q, w_kx, w_qx, proj_w, proj_b, memory_len, _dt_mm="float32r", _trace=False):
    from concourse.bass_utils import run_bass_kernel_spmd

    k = np.asarray(k, dtype=np.float32)
    q = np.asarray(q, dtype=np.float32)
    w_kx = np.asarray(w_kx, dtype=np.float32)
    w_qx = np.asarray(w_qx, dtype=np.float32)
    proj_w = np.asarray(proj_w, dtype=np.float32)
    proj_b = np.asarray(proj_b, dtype=np.float32)
    memory_len = np.asarray(memory_len)

    # live kk-tiles per batch (rows >= memory_len are exact zeros), and a
    # batch->core assignment pairing large with small so the uniform SPMD
    # slot sizes (T0, T1) are minimal.
    tcount = np.maximum(1, -(-np.asarray(memory_len, np.int64) // P))
    order = np.argsort(-tcount, kind="stable")
    slot0 = order[:NCORES]
    slot1 = order[2 * NCORES - 1:NCORES - 1:-1]   # reversed smallest half
    T0, T1 = int(tcount[slot0].max()), int(tcount[slot1].max())
    perm = np.empty((NCORES, BPC), np.int64)
    perm[:, 0] = slot0
    perm[:, 1] = slot1
    key = (_dt_mm, T0, T1)
    try:
        if key not in _compiled:
            _compiled[key] = _build_kernel(_dt_mm, (T0, T1))
        nc = _compiled[key]
    except Exception:
        if _dt_mm == "float32":
            raise
        # fall back to plain fp32 matmuls if this environment rejects f32r
        _dt_mm = "float32"
        key = (_dt_mm, T0, T1)
        if key not in _compiled:
            _compiled[key] = _build_kernel(_dt_mm, (T0, T1))
        nc = _compiled[key]
    if _dt_mm == "bf16":
        import ml_dtypes
        cast = lambda a: a.astype(ml_dtypes.bfloat16)
    else:
        cast = lambda a: a

    # ---- host-side sharding / preprocessing ----
    scale = np.float32(1.0 / math.sqrt(H))
    kT = cast(np.ascontiguousarray(k.transpose(0, 2, 1)))    # [B, e, s]
    qT = cast(np.ascontiguousarray(q.transpose(0, 2, 1)))
    head = np.arange(B) % NH
    def sbuf_layout(w):  # [B, e, h] -> [B, p, t*h] (partition-major e-tiles)
        return np.ascontiguousarray(
            w.reshape(B, NT, P, H).transpose(0, 2, 1, 3).reshape(B, P, NT * H))
    wk = cast(sbuf_layout(w_kx[head]))
    wq = cast(sbuf_layout(w_qx[head] * scale))
    pfT = np.ascontiguousarray(proj_w.reshape(EMB, NH, H).sum(axis=1).T)  # [h, e]
    kk = np.arange(SEQ)
    bias = np.where(kk[None, :] < memory_len[:, None], 0.0, -10000.0).astype(np.float32)
    mb = np.ascontiguousarray(bias.reshape(B, NT, P).transpose(0, 2, 1))  # [B, p, t]

    Ts = (T0, T1)
    in_maps = []
    for c in range(NCORES):
        bs = perm[c]
        m = {"qT": qT[bs], "wk": wk[bs], "wq": wq[bs],
             "pfT": pfT, "mb": mb[bs], "ident": np.eye(P, dtype=np.float32)}
        for j in range(BPC):
            m[f"kT{j}"] = np.ascontiguousarray(kT[bs[j]][:, :Ts[j] * P])
        in_maps.append(m)

    try:
        res = run_bass_kernel_spmd(nc, in_maps, list(range(NCORES)), trace=_trace)
    except Exception:
        if _dt_mm == "float32":
            raise
        fkey = ("float32", T0, T1)
        if fkey not in _compiled:
            _compiled[fkey] = _build_kernel("float32", (T0, T1))
        res = run_bass_kernel_spmd(_compiled[fkey], in_maps,
                                   list(range(NCORES)), trace=_trace)
    kernel.last_results = res
    kernel.last_nc = nc

    # ---- host-side unshard / epilogue ----
    out = np.empty((B, SEQ, EMB), np.float32)
    attn_base = np.empty((B, SEQ, SEQ), np.float32)
    projfold = pfT.T  # [e, h]
    for c in range(NCORES):
        oo = res.results[c]["oo"]   # [BPC, q, e]
        for j in range(BPC):
            b = int(perm[c][j])
            E_T = res.results[c][f"eo{j}"]   # [T*P, q] live rows only
            if memory_len[b] == 0:
                # fully masked: reference softmax is uniform 1/SEQ
                kx = k[b] @ w_kx[b % NH]
                attn_base[b] = np.float32(1.0 / SEQ)
                ob = np.broadcast_to(kx.mean(axis=0, dtype=np.float64).astype(np.float32), (SEQ, H))
                out[b] = ob @ projfold.T + proj_b[None, :]
                continue
            S = E_T.sum(axis=0)                         # [q]
            W = E_T.shape[0]
            attn_base[b, :, :W] = (E_T / S[None, :]).T
            attn_base[b, :, W:] = 0.0
            out[b] = oo[j] * (np.float32(1.0) / S)[:, None] + proj_b[None, :]
    attn = np.tile(attn_base, (NH, 1, 1))
    return out, attn
